# revision 1
# baseline (speedup 1.0000x reference)
"""YOLOv3-style detection decode on 8 Trainium2 NeuronCores (pure batch data-parallel).

Contract: kernel(**inputs) takes the FULL inputs from setup_inputs() and returns
the FULL output of reference(). Internally: batch dim 32 is sharded 4-per-core
across 8 cores. Only the 15 used channels (3 anchors x ch 0-4 of each 85-wide
block) are shipped per core, pre-packed host-side into the output's AoS row
order so the device kernel does the decode math (threshold mask, grid offset,
exp, anchor scaling, batch-index fill) with fully contiguous DMAs.
"""
import sys

sys.path.insert(0, "/opt/trn_rl_repo")

import numpy as np

N_CORES = 8
B_TOTAL = 32
B_PER_CORE = B_TOTAL // N_CORES
IMG = 416.0

# (grid size, padded per-partition floats F, anchors)  -- order of sections
# inside each per-batch span of the per-core packed tensor.
ANCHORS = {
    13: np.array([[116.0, 90.0], [156.0, 198.0], [373.0, 326.0]], np.float32),
    26: np.array([[30.0, 61.0], [62.0, 45.0], [59.0, 119.0]], np.float32),
    52: np.array([[10.0, 13.0], [16.0, 30.0], [33.0, 23.0]], np.float32),
}
HEADS = [
    # (grid H, F = padded floats/partition for one batch-section)
    (52, 320),   # 52*52*15 = 40560 <= 128*320 = 40960
    (26, 80),    # 26*26*15 = 10140 <= 128*80  = 10240
    (13, 20),    # 13*13*15 = 2535  <= 128*20  = 2560
]
SPAN = sum(f for _, f in HEADS)          # 420 floats per batch-section
F_TOTAL = SPAN * B_PER_CORE              # 1680
T_TOTAL = F_TOTAL // 5                   # 336 rows per partition
T_SPAN = SPAN // 5                       # 84 rows per batch-section


def _build_constants():
    """Compact constants: A2 [128, 2*T_SPAN] (grid col,row per output row),
    S4 [128, 4*T_SPAN] (scales t,t,aw,ah per output row)."""
    a_cols = []
    s_cols = []
    for H, F in HEADS:
        t = IMG / H
        anc = ANCHORS[H]
        n_rows = F // 5 * 128
        n_valid = H * H * 3
        r = np.arange(n_rows)
        pos = r // 3
        a = r % 3
        valid = r < n_valid
        A = np.zeros((n_rows, 2), np.float32)
        S = np.zeros((n_rows, 4), np.float32)
        A[valid, 0] = (pos % H)[valid]
        A[valid, 1] = (pos // H)[valid]
        S[valid, 0] = t
        S[valid, 1] = t
        S[valid, 2] = anc[a[valid], 0]
        S[valid, 3] = anc[a[valid], 1]
        a_cols.append(A.reshape(128, -1))
        s_cols.append(S.reshape(128, -1))
    return np.concatenate(a_cols, axis=1), np.concatenate(s_cols, axis=1)


_A_CONST, _S_CONST = _build_constants()
_CS16 = np.concatenate([_A_CONST, _S_CONST], axis=1).astype(np.float16)

_STATE = None


def _build_program():
    """Raw Bacc program with manual semaphores.

    Asymmetric software pipeline: section b0 (small, lands first) is decoded
    while sections b1-3 stream in, overlapping DMA latency with compute.
    Engines: Sync(SP) = input + output DMAs, Scalar(ACT) = exp + batch-index
    fills, Vector(DVE) = mask/grid-add/scale/mask-mult, PE = final completion
    wait (it sits last in the NEFF exit ring).  Compact constants
    (grid col/row, per-row scales, thresh, batch idx, zero bias) ride in one
    [128, 510] tensor "dcs".  Same-engine RAW hazards are synchronized by
    self-semaphores (producer increments at retire, consumer waits) because
    the DVE pipeline does not order reads of one instruction after writes of
    the previous one.
    """
    import concourse.bass as bass
    import concourse.bacc as bacc
    from concourse import mybir

    # Skip the Bass-constructor all-engine barrier (~0.8us): nothing in this
    # kernel reads the framework const APs (exp bias uses our own zero col).
    _orig_barrier = bass.Bass.all_engine_barrier
    bass.Bass.all_engine_barrier = lambda self, *a, **k: None
    try:
        nc = bacc.Bacc("TRN2", target_bir_lowering=False, debug=False)
    finally:
        bass.Bass.all_engine_barrier = _orig_barrier
    f32 = mybir.dt.float32
    f16 = mybir.dt.float16
    op = mybir.AluOpType
    A_W = 2 * T_SPAN                       # 168
    S_W = 4 * T_SPAN                       # 336
    HDR = 2 + B_PER_CORE                   # thresh | bvals | zero, in din
    IN = nc.dram_tensor("din", [128, HDR + F_TOTAL], f32, kind="ExternalInput")
    CS = nc.dram_tensor("dcs", [128, A_W + S_W], f16, kind="ExternalInput")
    OUT = nc.dram_tensor("dout", [128, F_TOTAL], f32, kind="ExternalOutput")

    tIN = nc.alloc_sbuf_tensor("tin", [128, HDR + F_TOTAL], f32)
    tZ = nc.alloc_sbuf_tensor("tz", [128, F_TOTAL], f32)
    tCS = nc.alloc_sbuf_tensor("tcs", [128, A_W + S_W], f16)
    tM = nc.alloc_sbuf_tensor("tm", [128, T_TOTAL], f32)

    s_cs = nc.alloc_semaphore("s_cs")      # constants DMA
    s_b0 = nc.alloc_semaphore("s_b0")      # input section b0 DMA
    s_p1 = nc.alloc_semaphore("s_p1")      # input [420:1050) DMA
    s_p2 = nc.alloc_semaphore("s_p2")      # input [1050:1680) DMA
    s_act = nc.alloc_semaphore("s_act")    # exps retired
    s_p = nc.alloc_semaphore("s_p")        # DVE isgt/add retired
    s_q = nc.alloc_semaphore("s_q")        # DVE mulS retired
    s_dve = nc.alloc_semaphore("s_dve")    # DVE mulM retired
    s_c = nc.alloc_semaphore("s_c")        # ACT c0-fills retired
    s_out = nc.alloc_semaphore("s_out")

    TAIL = 84                      # cols in the final (small) out-DMA
    T0 = T_SPAN                    # rows of section b0
    B3 = B_PER_CORE - 1

    dat = tIN.ap()[:, HDR:]
    inr = dat.rearrange("p (t c) -> p t c", c=5)           # [128,336,5]
    zr = tZ.ap().rearrange("p (t c) -> p t c", c=5)
    in4 = dat.rearrange("p (b t c) -> p b t c", b=B_PER_CORE, c=5)
    z4 = tZ.ap().rearrange("p (b t c) -> p b t c", b=B_PER_CORE, c=5)
    aT = tCS.ap()[:, 0:A_W].rearrange("p (t c) -> p t c", c=2)
    sT = tCS.ap()[:, A_W : A_W + S_W].rearrange("p (t c) -> p t c", c=4)
    thr = tIN.ap()[:, 0:1]
    zbias = tIN.ap()[:, HDR - 1 : HDR]
    bval = lambda b: tIN.ap()[:, 1 + b : 2 + b]

    # --- input DMAs balanced across the two HWDGE rings: the b1-3 bulk is
    # split so its halves transfer in parallel on both rings
    B0E = HDR + SPAN
    MID = B0E + SPAN
    nc.sync.dma_start(tIN.ap()[:, :B0E], IN.ap()[:, :B0E]).then_inc(s_b0, 16)
    nc.sync.dma_start(
        tIN.ap()[:, B0E:MID], IN.ap()[:, B0E:MID]
    ).then_inc(s_p1, 16)
    nc.scalar.dma_start(tCS.ap(), CS.ap()).then_inc(s_cs, 16)
    nc.scalar.dma_start(
        tIN.ap()[:, MID:], IN.ap()[:, MID:]
    ).then_inc(s_p2, 16)

    # --- ACT: exps per chain, then c0 fills
    # s_act: exp0=1 exp1=2 exp23=3
    nc.scalar.wait_ge(s_b0, 16)
    nc.scalar.activation(
        zr[:, :T0, 3:5], inr[:, :T0, 3:5],
        mybir.ActivationFunctionType.Exp, bias=zbias,
    ).then_inc(s_act, 1)
    nc.scalar.wait_ge(s_p1, 16)
    nc.scalar.activation(
        zr[:, T0 : 2 * T0, 3:5], inr[:, T0 : 2 * T0, 3:5],
        mybir.ActivationFunctionType.Exp, bias=zbias,
    ).then_inc(s_act, 1)
    nc.scalar.wait_ge(s_p2, 16)
    nc.scalar.activation(
        zr[:, 2 * T0 :, 3:5], inr[:, 2 * T0 :, 3:5],
        mybir.ActivationFunctionType.Exp, bias=zbias,
    ).then_inc(s_act, 1)

    def c0_fill(b, pwait):
        nc.scalar.wait_ge(s_p, pwait)
        sec = tZ.ap()[:, b * SPAN : (b + 1) * SPAN].rearrange(
            "p (t c) -> p t c", c=5
        )
        nc.scalar.activation(
            sec[:, :, 0],
            tM.ap()[:, b * T_SPAN : (b + 1) * T_SPAN],
            mybir.ActivationFunctionType.Copy,
            scale=bval(b),
        ).then_inc(s_c, 1)

    c0_fill(0, 1)
    c0_fill(1, 3)
    c0_fill(2, 5)
    c0_fill(3, 5)

    # --- DVE: three chains {b0} {b1} {b2,b3}
    # s_p: isgt0=1 add0=2 isgt1=3 add1=4 isgt23=5 add23=6
    # s_q: mulS k ; s_dve: mulM k   (k = 1,2,3)
    def chain(k, bs, be, ts, te, s_in, first):
        nbs = be - bs
        nc.vector.wait_ge(s_in, 16)
        nc.vector.tensor_scalar(
            tM.ap()[:, ts:te], inr[:, ts:te, 0], thr, None, op.is_gt
        ).then_inc(s_p, 1)
        if first:
            nc.vector.wait_ge(s_cs, 16)
        nc.vector.tensor_tensor(
            z4[:, bs:be, :, 1:3], in4[:, bs:be, :, 1:3],
            aT.unsqueeze(1).broadcast_to((128, nbs, T_SPAN, 2)), op.add,
        ).then_inc(s_p, 1)
        nc.vector.wait_ge(s_act, k)
        nc.vector.wait_ge(s_p, 2 * k)
        nc.vector.tensor_tensor(
            z4[:, bs:be, :, 1:5], z4[:, bs:be, :, 1:5],
            sT.unsqueeze(1).broadcast_to((128, nbs, T_SPAN, 4)), op.mult,
        ).then_inc(s_q, 1)
        nc.vector.wait_ge(s_q, k)
        for ms, me in (
            [(ts, te)] if be - bs == 1 else [(ts, ts + T0), (ts + T0, te)]
        ):
            m4 = tM.ap()[:, ms:me].unsqueeze(-1).broadcast_to(
                (128, me - ms, 4)
            )
            nc.vector.tensor_tensor(
                zr[:, ms:me, 1:5], zr[:, ms:me, 1:5], m4, op.mult
            ).then_inc(s_dve, 1)

    chain(1, 0, 1, 0, T0, s_b0, True)
    chain(2, 1, 2, T0, 2 * T0, s_p1, False)
    chain(3, 2, 4, 2 * T0, T_TOTAL, s_p2, False)

    # --- output DMAs on SP: per-chain, small tail last
    nc.sync.wait_ge(s_dve, 1)
    nc.sync.wait_ge(s_c, 1)
    nc.sync.dma_start(OUT.ap()[:, :SPAN], tZ.ap()[:, :SPAN]).then_inc(s_out, 16)
    nc.sync.wait_ge(s_dve, 2)
    nc.sync.wait_ge(s_c, 2)
    nc.sync.dma_start(
        OUT.ap()[:, SPAN : 2 * SPAN], tZ.ap()[:, SPAN : 2 * SPAN]
    ).then_inc(s_out, 16)
    nc.sync.wait_ge(s_dve, 3)
    nc.sync.wait_ge(s_c, 3)
    nc.sync.dma_start(
        OUT.ap()[:, 2 * SPAN : 3 * SPAN], tZ.ap()[:, 2 * SPAN : 3 * SPAN]
    ).then_inc(s_out, 16)
    nc.sync.wait_ge(s_dve, 4)
    nc.sync.wait_ge(s_c, 4)
    nc.sync.dma_start(
        OUT.ap()[:, 3 * SPAN :], tZ.ap()[:, 3 * SPAN :]
    ).then_inc(s_out, 16)

    # completion wait on the idle PE engine (last in the exit ring)
    nc.tensor.wait_ge(s_out, 64)
    nc.compile()
    return nc


def _pack_head(arr, H):
    """[B, 255, H, W] full head tensor -> per-batch padded sections.

    Returns [B, 128, F] float32: batch b's section as the [128, F] block.
    """
    B = arr.shape[0]
    F = dict(HEADS)[H]
    hw = H * H
    # channels 85*a + c for a in 0..2, c in 0..4  -> [B, 3, 5, HW]
    sel = arr.reshape(B, 3, 85, hw)[:, :, 0:5, :]
    # -> [B, HW, 3, 5] row-major AoS (pos, anchor, channel)
    aos = np.ascontiguousarray(sel.transpose(0, 3, 1, 2))
    flat = aos.reshape(B, hw * 15)
    out = np.zeros((B, 128 * F), np.float32)
    out[:, : hw * 15] = flat
    return out.reshape(B, 128, F)


def kernel(output_13, output_26, output_52, thresh):
    global _STATE
    if _STATE is None:
        _STATE = _build_program()
    nc = _STATE

    from concourse.bass_utils import run_bass_kernel_spmd

    heads_np = {13: np.asarray(output_13, np.float32),
                26: np.asarray(output_26, np.float32),
                52: np.asarray(output_52, np.float32)}
    thr = float(np.asarray(thresh))

    packed = {H: _pack_head(heads_np[H], H) for H, _ in HEADS}

    in_maps = []
    for core in range(N_CORES):
        secs = []
        for b in range(B_PER_CORE):
            bg = core * B_PER_CORE + b
            for H, F in HEADS:
                secs.append(packed[H][bg])
            # (concatenated below along the free axis)
        cst = np.zeros((128, 2 + B_PER_CORE), np.float32)
        cst[:, 0] = thr
        for b in range(B_PER_CORE):
            cst[:, 1 + b] = float(core * B_PER_CORE + b)
        din = np.concatenate([cst] + secs, axis=1)
        in_maps.append({"din": din, "dcs": _CS16})

    res = run_bass_kernel_spmd(nc, in_maps, core_ids=list(range(N_CORES)))

    # Unshard: output rows are [head13 | head26 | head52], each head
    # batch-major with H*H*3 rows per batch.
    n_rows = sum(H * H * 3 for H, _ in HEADS) * B_TOTAL
    out = np.empty((n_rows, 5), np.float32)
    head_off = 0
    for H in (13, 26, 52):
        F = dict(HEADS)[H]
        rows_per_b = H * H * 3
        sec_off = 0
        for HH, FF in HEADS:
            if HH == H:
                break
            sec_off += FF
        for core in range(N_CORES):
            o = res.results[core]["dout"]
            for b in range(B_PER_CORE):
                bg = core * B_PER_CORE + b
                sec = o[:, b * SPAN + sec_off : b * SPAN + sec_off + F]
                rows = sec.reshape(-1)[: rows_per_b * 5].reshape(rows_per_b, 5)
                out[head_off + bg * rows_per_b : head_off + (bg + 1) * rows_per_b] = rows
        head_off += rows_per_b * B_TOTAL
    return out

